# revision 1
# baseline (speedup 1.0000x reference)
"""Trainium2 Bass kernel for nn_Memory_22548578304755 (scatter_memory).

Computes: mean_b [ -log_softmax(mask(inputs @ features.T / temp))[b, indices[b]] ]

Strategy (8 NeuronCores, SPMD):
  - The host sorts the feature bank by camera id and deals each camera's
    rows round-robin across the 8 cores, padding each camera's per-core
    range to a common width ceil(N_c/8) with zero rows. Every core then
    holds the SAME column layout (camera c at columns [off_c, off_c+M_c)),
    so a single SPMD program serves all cores and the intra-camera mask
    disappears: exp-sums are accumulated per camera-pure column segment
    and the host picks each sample's own-camera denominator. Zero-pad
    columns contribute exp(-K_SHIFT) ~ 1e-44, i.e. nothing.
  - Features and the query block are quantized to fp8e4m3 (features
    scaled by 64), enabling DoubleRow matmuls: one instruction contracts
    2x128 rows, halving PE instruction count; per 500-col group only 8
    matmuls + 1-2 exp-activations remain.
  - The shard is laid out group-major ([NG, 128, KC, 500]) so every 1MB
    group DMA is a fully contiguous 8KB-per-partition read; group DMAs
    round-robin across DMA queues with a bounded lookahead.
  - Each exp-activation (ScalarE, scale=1/64 descale, bias=-K_SHIFT)
    accumulates its segment into a partials column; the host combines the
    8 cores' partials (cross-device logsumexp) and subtracts exact fp64
    target scores for the final scalar.
"""

import sys

import numpy as np

sys.path.insert(0, "/opt/trn_rl_repo")

import ml_dtypes  # noqa: E402

import concourse.bacc as bacc  # noqa: E402
import concourse.mybir as mybir  # noqa: E402
from concourse.tile import TileContext  # noqa: E402
from concourse.bass_utils import run_bass_kernel_spmd  # noqa: E402

B = 64
N = 100000
D = 2048
NCAMS = 8
TEMP = 0.07
NCORES = 8

K_SHIFT = 100.0  # shift so exp never overflows (max score ~64)
FEAT_SCALE = 64.0  # fp8 feature pre-scale (power of 2)

KC = D // 128  # 16 contraction chunks of 128 (8 DoubleRow pairs)
N_MM = 500  # group width (one PSUM bank)
CG = "mixed"  # groups per DMA chunk: int, or "mixed" = [1,2,2,...,2,1]
LOOKAHEAD = 6  # chunk DMAs in flight ahead of compute
FEAT_BUFS = 7

PLAN = ("sync",)  # single HW DGE queue saturates the per-core HBM path


def _schedule(counts):
    """Uniform cross-core column layout + camera-pure activation pieces.

    counts: per-camera row counts over the full bank.
    Returns (widths, pieces) where widths[g] is group g's column count and
    pieces is a tuple of (group, a, b, cam) activation sub-ranges.
    """
    M_c = [(int(c) + NCORES - 1) // NCORES for c in counts]
    off = np.concatenate([[0], np.cumsum(M_c)])
    M_pad = int(off[-1])
    ngf = M_pad // N_MM
    extra = M_pad - ngf * N_MM
    if extra and extra <= 512 - N_MM:
        widths = [N_MM] * (ngf - 1) + [N_MM + extra]
    else:
        widths = [N_MM] * ngf + ([extra] if extra else [])
    if widths[0] >= 500:
        widths = [250, widths[0] - 250] + widths[1:]  # fast pipeline fill
    if widths[-1] >= 500:
        widths = widths[:-1] + [widths[-1] - 250, 250]  # small final quantum
    starts = np.concatenate([[0], np.cumsum(widths)])
    cuts = sorted(set([int(v) for v in starts] + [int(o) for o in off]))
    pieces = []
    for lo, hi in zip(cuts, cuts[1:]):
        g = int(np.searchsorted(starts, lo, side="right") - 1)
        cam = int(np.searchsorted(off, lo, side="right") - 1)
        pieces.append((g, lo - int(starts[g]), hi - int(starts[g]), cam))
    return tuple(widths), tuple(pieces), off[:-1], M_pad


def _chunks(widths, cg):
    """Partition group list into chunks. cg: int group count or "mixed"
    (1-group first/last chunks for fast pipeline fill/drain, 2 elsewhere)."""
    ng = len(widths)
    if cg == "mixed":
        sizes = [1, 1]
        while ng - sum(sizes) > 2:
            sizes.append(2)
        sizes += [1] * (ng - sum(sizes))
    else:
        sizes = []
        while sum(sizes) < ng:
            sizes.append(min(cg, ng - sum(sizes)))
    out = []
    g = 0
    for k in sizes:
        out.append((g, k, sum(widths[g + i] for i in range(k))))
        g += k
    return out


def build_nc(widths, pieces, plan=PLAN, cg=CG):
    """Build the single-core Bass program (identical across the 8 cores)."""
    dt = mybir.dt
    DR = mybir.MatmulPerfMode.DoubleRow
    nc = bacc.Bacc()

    ng = len(widths)
    npieces = len(pieces)
    chunks = _chunks(widths, cg)

    featCk = [
        nc.declare_dram_parameter(f"feat{ci}", [128, KC, w], dt.float8e4, False)
        for ci, (_g0, _k, w) in enumerate(chunks)
    ]
    inp8 = nc.declare_dram_parameter("inp8", [128, KC, B], dt.float8e4, False)
    init = nc.declare_dram_parameter("init", [B, 1 + npieces], dt.float32, False)
    out = nc.declare_dram_parameter("out", [B, npieces], dt.float32, True)

    by_group = [[] for _ in range(ng)]
    for i, (g, a, b, _cam) in enumerate(pieces):
        by_group[g].append((a, b, i))

    with TileContext(nc) as tc:
        with (
            tc.tile_pool(name="feat", bufs=FEAT_BUFS) as featp,
            tc.tile_pool(name="small", bufs=1) as smallp,
            tc.tile_pool(name="scratch", bufs=3) as scrp,
            tc.tile_pool(name="psum", bufs=8, space="PSUM") as psump,
        ):
            inp_t = smallp.tile([128, KC, B], dt.float8e4)
            init_t = smallp.tile([B, 1 + npieces], dt.float32)
            nc.scalar.dma_start(inp_t[:], inp8[:, :, :])
            nc.scalar.dma_start(init_t[:], init[:, :])
            nbias = init_t[:, 0:1]
            partials = init_t[:, 1 : 1 + npieces]

            queues = {"sync": nc.sync, "gpsimd": nc.gpsimd, "scalar": nc.scalar}
            fts = [None] * len(chunks)
            wmax = max(w for _g0, _k, w in chunks)

            def issue(ci):
                if ci >= len(chunks):
                    return
                g0c, _k, w = chunks[ci]
                q = queues[plan[ci % len(plan)]]
                fta = featp.tile([128, KC // 2, wmax], dt.float8e4, tag="fta")
                ftb = featp.tile([128, KC // 2, wmax], dt.float8e4, tag="ftb")
                h = KC // 2
                if g0c % 2 == 0:  # first group ascends k: low half first
                    q.dma_start(fta[:, :, :w], featCk[ci][:, :h, :])
                    q.dma_start(ftb[:, :, :w], featCk[ci][:, h:, :])
                else:  # first group descends k: high half first
                    q.dma_start(ftb[:, :, :w], featCk[ci][:, h:, :])
                    q.dma_start(fta[:, :, :w], featCk[ci][:, :h, :])
                fts[ci] = (fta, ftb)

            for ci in range(min(LOOKAHEAD, len(chunks))):
                issue(ci)

            g2c = {}
            for ci, (g0, k, _w) in enumerate(chunks):
                for i in range(k):
                    g2c[g0 + i] = (ci, i)
            for g in range(ng):
                w = widths[g]
                ci, gl = g2c[g]
                g0 = chunks[ci][0]
                fta, ftb = fts[ci]
                co = sum(widths[g0 + i] for i in range(gl))
                ps = psump.tile([B, 512], dt.float32, tag="ps")
                # k-snake: alternate contraction order so consecutive groups
                # share the boundary weight load.
                korder = range(KC // 2) if g % 2 == 0 else range(KC // 2 - 1, -1, -1)
                for j, kk in enumerate(korder):
                    half, kl = (fta, kk) if kk < KC // 4 else (ftb, kk - KC // 4)
                    nc.tensor.matmul(
                        ps[:, :w],
                        inp_t[:, 2 * kk : 2 * kk + 2, :],
                        half[:, 2 * kl : 2 * kl + 2, co : co + w],
                        start=(j == 0),
                        stop=(j == KC // 2 - 1),
                        perf_mode=DR,
                    )
                for a, b, i in by_group[g]:
                    ex = scrp.tile([B, 512], dt.float32, tag="ex")
                    nc.scalar.activation(
                        ex[:, : b - a],
                        ps[:, a:b],
                        mybir.ActivationFunctionType.Exp,
                        bias=nbias[:, :],
                        scale=1.0 / FEAT_SCALE,
                        accum_out=partials[:, i : i + 1],
                    )
                if gl == chunks[ci][1] - 1:
                    issue(ci + LOOKAHEAD)

            nc.scalar.dma_start(out[:, :], partials[:])
    nc.finalize()
    return nc


def _prep_host(inputs, features, indices, camids, camids_batch, cg=CG):
    """Host-side shard prep. Returns dict with in_maps, schedule, targets."""
    f8 = ml_dtypes.float8_e4m3
    x = np.asarray(inputs, np.float32) / TEMP  # [B, D]
    cb = np.asarray(camids_batch).astype(np.int64)
    cn = np.asarray(camids).astype(np.int64)
    idx = np.asarray(indices).astype(np.int64)
    feats = np.asarray(features, np.float32)

    counts = np.bincount(cn, minlength=NCAMS)
    widths, pieces, off, M_pad = _schedule(counts)
    ngf = sum(1 for w in widths if w == N_MM)
    extra = M_pad - ngf * N_MM

    # inp8[p, k, b] = x[b, k*128+p]
    inp8 = np.ascontiguousarray(x.T.reshape(KC, 128, B).transpose(1, 0, 2).astype(f8))

    # exact target scores on host (fp64)
    tsel = np.einsum("bd,bd->b", x.astype(np.float64), feats[idx].astype(np.float64))

    # quantized, transposed bank with a zero column at index N for padding
    F8 = np.empty((D, N + 1), f8)
    F8[:, :N] = (feats.T * FEAT_SCALE).astype(f8)
    F8[:, N] = 0

    # deal each camera's rows round-robin across cores at identical offsets
    order = np.argsort(cn, kind="stable")
    bounds = np.concatenate([[0], np.cumsum(counts)])
    colmap = np.full((NCORES, M_pad), N, np.int64)
    for c in range(NCAMS):
        rc = order[bounds[c] : bounds[c + 1]]
        j = np.arange(len(rc))
        colmap[j % NCORES, off[c] + j // NCORES] = rc

    chunks = _chunks(widths, cg)
    cb_cols = np.concatenate([[0], np.cumsum(widths)])
    init0 = np.zeros((B, 1 + len(pieces)), np.float32)
    init0[:, 0] = -K_SHIFT
    in_maps = []
    for k in range(NCORES):
        fr = F8[:, colmap[k]].reshape(KC, 128, M_pad)
        m = {"inp8": inp8, "init": init0}
        for ci, (g0, _kk, w) in enumerate(chunks):
            a = int(cb_cols[g0])
            m[f"feat{ci}"] = np.ascontiguousarray(fr[:, :, a : a + w].transpose(1, 0, 2))
        in_maps.append(m)
    return {
        "in_maps": in_maps,
        "tsel": tsel,
        "cb": cb,
        "widths": widths,
        "pieces": pieces,
        "cg": cg,
    }


def _combine_host(results, prep):
    """Cross-core logsumexp combine -> final scalar."""
    raw = np.stack([r["out"] for r in results]).astype(np.float64).sum(axis=0)  # [B, P]
    Dcam = np.zeros((B, NCAMS))
    for i, (_g, _a, _b, cam) in enumerate(prep["pieces"]):
        Dcam[:, cam] += raw[:, i]
    den = Dcam[np.arange(B), prep["cb"]]
    nll = np.log(den) + K_SHIFT - prep["tsel"]
    return np.float32(nll.mean())


_NC_CACHE = {}


def _get_nc(widths, pieces, plan=PLAN, cg=CG):
    key = (widths, pieces, plan, cg)
    if key not in _NC_CACHE:
        _NC_CACHE[key] = build_nc(widths, pieces, plan, cg)
    return _NC_CACHE[key]


def run_device(prep, plan=PLAN, **kwargs):
    nc = _get_nc(prep["widths"], prep["pieces"], plan, prep["cg"])
    return run_bass_kernel_spmd(
        nc, prep["in_maps"], core_ids=list(range(len(prep["in_maps"]))), **kwargs
    )


def kernel(inputs, features, indices, camids, camids_batch):
    prep = _prep_host(inputs, features, indices, camids, camids_batch)
    res = run_device(prep)
    return _combine_host(res.results, prep)

